# revision 15
# baseline (speedup 1.0000x reference)
"""DeepFM (embedding_lookup) Trainium2 kernel.

Pure data parallel over the flattened (B*G) row dimension across 8
NeuronCores. Per core (16K rows, 8 blocks of 2048):

- Embedding lookup via gpsimd dma_gather (mlp ucode), one call per
  (field, block): 2048 int16 indices gather 256B rows (128 bf16:
  [emb2d(16) | emb1d(1) | sum(emb2d^2)(1) | pad]) from a per-field DRAM
  table. Descgen runs on the Q7 pair (2q, 2q+1) selected by queue_num at
  ~8ns/idx/pair (probe-verified; invariant to call size and elem size), so
  queue_num = field%4 works all four pairs. Sustaining that requires:
  (a) the Pool queue holds ONLY gathers - idxs preloaded via nc.sync (a
      small contiguous idx0 input first so block 0 never waits the 4MB
      bulk load; strided idx DMAs spray thousands of descriptors that
      steal SDMA slots from the gathers - keep them contiguous),
      num_idxs hoisted to one register, all other DMAs on HWDGE;
  (b) two DMASW sem lanes per queue (q and q+4, alternating) - Tile
      serializes reuse of a lane, so one lane per queue caps each queue at
      one call in flight and adds the ~4us completion tail to every call;
  (c) the DVE queue holds ONLY the gather compaction copies - PSUM->SBUF
      xT copies ride ScalarE(Copy), else compacts queue behind the prior
      block's DNN copies (in-order DVE) and stall gathers ~20us per block.
  Steady state: 16.9us per 4-call group = the pure descgen floor.
- DVE compacts the 18 useful bf16 of each row into [128, chunk, field, 18].
- The DNN runs transposed (features on partitions, rows on free dim) in
  bf16 (4x PE rate vs fp32r): PE transposes each 256-row tile (DVE
  StreamTranspose is per-32x32-block only - can't replace it), fp32 PSUM
  accumulate, PSUM->SBUF copies fuse bias+relu on ScalarE.

FM algebra folded into the output matmul:
  out = relu(pe @ wo + bo),  pe = [fm1, fm2, h2]
  fm1 = sum_f e1d_f + n @ dense_w + dense_b      -> wo row 0 contributions:
        e1d columns of xT get weight wo[0]; (dense_w @ wo[0:1]) rides the nT
        K-chunk; dense_b*wo[0] folds into the output bias.
  fm2 = 0.5*(sum_d s1_d^2 - sum_f sq_f), s1 = sum_f e_f, sq_f = sum_d e_fd^2
        -> sq columns of xT get weight -0.5*wo[1]; s1 comes from a small
        selection matmul; s1^2 enters via lhsT = tile(0.5*wo[1]).

Measured (NTFF, 8 cores): 626us, rel err 4.3e-3 (gate 2e-2). Structure:
~21us head (mlp library IRAM load - floor) + 542us gather span (128 calls
x 16.9us/4 - the Q7 descgen floor) + ~65us tail (last blocks' DNN; PE is
~53us busy/block, likely HAM 4/8 clock throttle from bursty duty - tile-
stage pipelining and last-block splitting both measured neutral/negative).
The next lever would be the 8ns/idx ucode descgen itself (no Xtensa
toolchain in-container to rebuild).
"""

import sys

sys.path.insert(0, "/opt/trn_rl_repo")

from contextlib import ExitStack

import numpy as np
import ml_dtypes

import concourse.tile as tile
from concourse import bacc, bass, mybir
from concourse.bass_utils import run_bass_kernel_spmd
from concourse import library_config

BF16NP = np.dtype(ml_dtypes.bfloat16)

B, G = 128, 1024
F, V, D = 16, 10000, 16
FN = 13
H1, H2, OUT = 256, 128, 128
NCORES = 8
ROWS = B * G
R = ROWS // NCORES  # rows per core
AUGW = 18  # e(16) | e1d | sumsq
PADW = 128  # padded DRAM row in bf16 elems (256B, dma_gather elem quantum)
XW = F * AUGW  # 288 transposed feature rows

BLK_ROWS = 2048
NCHUNK = BLK_ROWS // 128  # 16 chunks of 128 rows per block
TPB = NCHUNK // 2  # 8 tiles of 256 rows per block

F32 = mybir.dt.float32
BF16 = mybir.dt.bfloat16
I16 = mybir.dt.int16
RELU = mybir.ActivationFunctionType.Relu
SQUARE = mybir.ActivationFunctionType.Square
COPYF = mybir.ActivationFunctionType.Copy

# Packed weights layout: name -> (col offset, n cols, n valid rows)
WOFFS = {
    "w1c0": (0, 256, 128), "w1c1": (256, 256, 128), "w1c2": (512, 256, 45),
    "sc0": (768, 16, 128), "sc1": (784, 16, 128), "sc2": (800, 16, 32),
    "w2c0": (816, 128, 128), "w2c1": (944, 128, 128),
    "woc0": (1072, 128, 128), "woc1": (1200, 128, 128), "woc2": (1328, 128, 45),
    "wsq": (1456, 128, 16), "wo2": (1584, 128, 128),
}
WPACK_COLS = 1712


def _pin_swdge_sems_to_queues():
    """Pin each Pool DMA's DMASW sem lane to queue_num + 4*(per-queue count%2):
    lanes alternate within a queue so Tile's serialize-on-lane-reuse allows TWO
    calls in flight per queue (descgen of call k+4 overlaps the drain/completion
    tail of call k). Lane mod 4 == queue keeps per-lane completion order = queue
    FIFO order, which the ucode's ring reclaim requires."""
    import concourse.tile_sem_assignment as tsa

    if getattr(tsa.TileClockTick, "_qpinned", False):
        return
    orig = tsa.TileClockTick._assign_tick
    counts = {}

    def patched(self, inst):
        if isinstance(inst, tsa.DMAInst) and inst.engine == mybir.EngineType.Pool:
            q = getattr(inst, "queue_num", 0) or 0
            c = counts.get(q, 0)
            counts[q] = c + 1
            self.next_sw_dma_idx = q + 4 * (c % 2)
        return orig(self, inst)

    tsa.TileClockTick._assign_tick = patched
    tsa.TileClockTick._qpinned = True


def build_program(nblk):
    _pin_swdge_sems_to_queues()
    nc = bacc.Bacc(
        "TRN2",
        target_bir_lowering=False,
        debug=False,
        num_swdge_queues=4,
        dynamic_dma_scratch_size=32768,
    )

    def din(name, shape, dt):
        return nc.dram_tensor(name, list(shape), dt, kind="ExternalInput").ap()

    r_rows = nblk * BLK_ROWS
    padbf = din("padbf", [F, V, PADW], BF16)
    idx16 = din("idx16", [128, F, nblk, 128], I16)
    idx0 = din("idx0", [128, F, 128], I16)
    nT = din("nT", [FN, r_rows], BF16)
    wpack = din("wpack", [128, WPACK_COLS], BF16)
    biasd = din("biasd", [128, 4], F32)
    identd = din("identd", [128, 128], BF16)
    outT = nc.dram_tensor("outT", [128, r_rows], F32, kind="ExternalOutput").ap()

    with tile.TileContext(nc) as tc, ExitStack() as ctx:
        singles = ctx.enter_context(tc.tile_pool(name="singles", bufs=1))
        gfp = ctx.enter_context(tc.tile_pool(name="gfp", bufs=10))
        gp = ctx.enter_context(tc.tile_pool(name="gather", bufs=2))
        op = ctx.enter_context(tc.tile_pool(name="outbuf", bufs=2))
        xs = ctx.enter_context(tc.tile_pool(name="xs", bufs=4))
        hs = ctx.enter_context(tc.tile_pool(name="hs", bufs=3))
        ps_xt = ctx.enter_context(tc.tile_pool(name="ps_xt", bufs=2, space="PSUM"))
        ps_h1 = ctx.enter_context(tc.tile_pool(name="ps_h1", bufs=2, space="PSUM"))
        ps_h2 = ctx.enter_context(tc.tile_pool(name="ps_h2", bufs=1, space="PSUM"))
        ps_wo = ctx.enter_context(tc.tile_pool(name="ps_wo", bufs=2, space="PSUM"))
        ps_s1 = ctx.enter_context(tc.tile_pool(name="ps_s1", bufs=1, space="PSUM"))

        nc.gpsimd.load_library(library_config.mlp)

        idx0s = singles.tile([128, F, 128], I16)
        nc.sync.dma_start(out=idx0s[:], in_=idx0[:])
        ident = singles.tile([128, 128], BF16)
        nc.sync.dma_start(out=ident[:], in_=identd[:])
        idxs = singles.tile([128, F, nblk, 128], I16)
        nc.sync.dma_start(out=idxs[:], in_=idx16[:])
        wpk = singles.tile([128, WPACK_COLS], BF16)
        nc.sync.dma_start(out=wpk[:], in_=wpack[:])
        bias = singles.tile([128, 4], F32)
        nc.sync.dma_start(out=bias[:], in_=biasd[:])
        nTs = singles.tile([FN, r_rows], BF16)
        nc.sync.dma_start(out=nTs[:], in_=nT[:])

        wt = {
            name: wpk[0:nrows, off : off + ncols]
            for name, (off, ncols, nrows) in WOFFS.items()
        }

        # Priming transpose: absorbs the ident-DMA dependency into PE program
        # order so later transposes carry only the gather wait (the S3_LW
        # slot fits a single sync wait).
        warm = ps_wo.tile([128, 128], BF16, space="PSUM", tag="pwo")
        nc.tensor.transpose(out=warm[:], in_=ident[:], identity=ident[:])
        nc.tensor.transpose(out=warm[:], in_=wpk[:, 0:128], identity=ident[:])
        scr = singles.tile([128, 1], F32, tag="scr")
        nc.scalar.activation(out=scr[:], in_=bias[:, 0:1], func=mybir.ActivationFunctionType.Copy)
        scr2 = singles.tile([FN, 1], BF16, tag="scr2")
        nc.vector.tensor_copy(out=scr2[:], in_=nTs[:, 0:1])

        nreg = nc.gpsimd.to_reg(BLK_ROWS)

        for b in range(nblk):
            blk = gp.tile([128, NCHUNK, F, AUGW], BF16)
            for f in range(F):
                gf = gfp.tile([128, NCHUNK, PADW], BF16, tag="gf")
                nc.gpsimd.dma_gather(
                    out_ap=gf[:],
                    in_ap=padbf[f],
                    idxs_ap=idx0s[:, f, :] if b == 0 else idxs[:, f, b, :],
                    num_idxs=BLK_ROWS,
                    num_idxs_reg=nreg,
                    elem_size=PADW,
                    queue_num=f % 4,
                    single_packet=False,
                )
                nc.vector.tensor_copy(out=blk[:, :, f, :], in_=gf[:, :, 0:AUGW])
            blkf = blk[:].rearrange("p c f j -> p c (f j)")  # [128, NCHUNK, 288]
            outb = op.tile([128, BLK_ROWS], F32)

            for t in range(TPB):
                rlo = b * BLK_ROWS + t * 256
                # ---- transpose x tile [256 rows, 288 cols] -> xT pieces ----
                pxt = ps_xt.tile([128, 768], BF16, space="PSUM")
                for ci in range(2):
                    ch = 2 * t + ci
                    for k in range(3):
                        lo = k * 128
                        hi = min(XW, lo + 128)
                        if k < 2:
                            dst = pxt[:, k * 256 + ci * 128 : k * 256 + (ci + 1) * 128]
                        else:
                            dst = pxt[0:32, 512 + ci * 128 : 512 + (ci + 1) * 128]
                        nc.tensor.transpose(
                            out=dst, in_=blkf[:, ch, lo:hi], identity=ident[:]
                        )
                xt0 = xs.tile([128, 256], BF16)
                xt1 = xs.tile([128, 256], BF16)
                xt2 = xs.tile([45, 256], BF16)
                nc.scalar.activation(out=xt0[:], in_=pxt[:, 0:256], func=COPYF)
                nc.scalar.activation(out=xt1[:], in_=pxt[:, 256:512], func=COPYF)
                nc.scalar.activation(out=xt2[0:32, :], in_=pxt[0:32, 512:768], func=COPYF)
                nc.scalar.activation(out=xt2[32:45, :], in_=nTs[:, rlo : rlo + 256], func=COPYF)

                # ---- s1 = sum_f e_f  (transposed: [16, 256]) ----
                ps1 = ps_s1.tile([16, 256], F32, space="PSUM")
                nc.tensor.matmul(ps1[:], wt["sc0"][:], xt0[:], start=True, stop=False)
                nc.tensor.matmul(ps1[:], wt["sc1"][:], xt1[:], start=False, stop=False)
                nc.tensor.matmul(ps1[:], wt["sc2"][:], xt2[0:32, :], start=False, stop=True)
                s1sq = xs.tile([16, 256], BF16)
                nc.scalar.activation(out=s1sq[:], in_=ps1[:], func=SQUARE)

                # ---- h1 = relu(x @ w1 + b1)  -> h1T [256, rows] in 2 halves ----
                ph1 = ps_h1.tile([128, 512], F32, space="PSUM")
                for mh in range(2):
                    dst = ph1[:, mh * 256 : (mh + 1) * 256]
                    ms = slice(mh * 128, (mh + 1) * 128)
                    nc.tensor.matmul(dst, wt["w1c0"][:, ms], xt0[:], start=True, stop=False)
                    nc.tensor.matmul(dst, wt["w1c1"][:, ms], xt1[:], start=False, stop=False)
                    nc.tensor.matmul(dst, wt["w1c2"][:, ms], xt2[:], start=False, stop=True)
                h1t = hs.tile([128, 512], BF16)
                nc.scalar.activation(out=h1t[:, 0:256], in_=ph1[:, 0:256], func=RELU, bias=bias[:, 0:1])
                nc.scalar.activation(out=h1t[:, 256:512], in_=ph1[:, 256:512], func=RELU, bias=bias[:, 1:2])

                # ---- h2 = relu(h1 @ w2 + b2) -> h2T [128, rows] ----
                ph2 = ps_h2.tile([128, 256], F32, space="PSUM")
                nc.tensor.matmul(ph2[:], wt["w2c0"][:], h1t[:, 0:256], start=True, stop=False)
                nc.tensor.matmul(ph2[:], wt["w2c1"][:], h1t[:, 256:512], start=False, stop=True)
                h2t = hs.tile([128, 256], BF16)
                nc.scalar.activation(out=h2t[:], in_=ph2[:], func=RELU, bias=bias[:, 2:3])

                # ---- out = relu(pe @ wo + bo') -> outT [128, rows] ----
                pwo = ps_wo.tile([128, 256], F32, space="PSUM")
                nc.tensor.matmul(pwo[:], wt["woc0"][:], xt0[:], start=True, stop=False)
                nc.tensor.matmul(pwo[:], wt["woc1"][:], xt1[:], start=False, stop=False)
                nc.tensor.matmul(pwo[:], wt["woc2"][:], xt2[:], start=False, stop=False)
                nc.tensor.matmul(pwo[:], wt["wsq"][:], s1sq[:], start=False, stop=False)
                nc.tensor.matmul(pwo[:], wt["wo2"][:], h2t[:], start=False, stop=True)
                nc.scalar.activation(
                    out=outb[:, t * 256 : (t + 1) * 256], in_=pwo[:], func=RELU, bias=bias[:, 3:4]
                )

            nc.sync.dma_start(
                out=outT[:, b * BLK_ROWS : (b + 1) * BLK_ROWS], in_=outb[:]
            )

    nc.compile()
    return nc


def prep_weights(inputs):
    emb1d = np.asarray(inputs["emb1d"], np.float32)
    emb2d = np.asarray(inputs["emb2d"], np.float32)
    dense_w = np.asarray(inputs["dense_w"], np.float32)
    dense_b = np.asarray(inputs["dense_b"], np.float32)
    w1 = np.asarray(inputs["w1"], np.float32)
    b1 = np.asarray(inputs["b1"], np.float32)
    w2 = np.asarray(inputs["w2"], np.float32)
    b2 = np.asarray(inputs["b2"], np.float32)
    wo = np.asarray(inputs["wo"], np.float32)
    bo = np.asarray(inputs["bo"], np.float32)

    padbf = np.zeros((F, V, PADW), BF16NP)
    padbf[:, :, 0:D] = emb2d.astype(BF16NP)
    padbf[:, :, D] = emb1d[:, :, 0].astype(BF16NP)
    padbf[:, :, D + 1] = (emb2d * emb2d).sum(2).astype(BF16NP)

    # x columns (f-major e layout) -> augmented-row column index
    cols = np.arange(F * D)
    fidx = (cols // D) * AUGW + (cols % D)
    w1aug = np.zeros((XW, H1), np.float32)
    w1aug[fidx] = w1[0 : F * D]
    w1c2 = np.zeros((45, H1), np.float32)
    w1c2[0:32] = w1aug[256:288]
    w1c2[32:45] = w1[F * D : F * D + FN]

    saug = np.zeros((XW, D), np.float32)
    saug[fidx, cols % D] = 1.0

    woaug = np.zeros((XW, OUT), np.float32)
    woaug[np.arange(F) * AUGW + D] = wo[0]  # e1d cols -> fm1_sparse * wo0
    woaug[np.arange(F) * AUGW + D + 1] = -0.5 * wo[1]  # sq cols -> -0.5*t2*wo1
    woc2 = np.zeros((45, OUT), np.float32)
    woc2[0:32] = woaug[256:288]
    woc2[32:45] = dense_w.reshape(FN, 1) @ wo[0:1]

    blocks = dict(
        w1c0=w1aug[0:128], w1c1=w1aug[128:256], w1c2=w1c2,
        sc0=saug[0:128], sc1=saug[128:256], sc2=saug[256:288],
        w2c0=w2[0:128], w2c1=w2[128:256],
        woc0=woaug[0:128], woc1=woaug[128:256], woc2=woc2,
        wsq=np.tile(0.5 * wo[1:2], (D, 1)),
        wo2=wo[2:130],
    )
    wpack = np.zeros((128, WPACK_COLS), BF16NP)
    for name, (off, ncols, nrows) in WOFFS.items():
        arr = blocks[name]
        assert arr.shape == (nrows, ncols), (name, arr.shape, (nrows, ncols))
        wpack[0:nrows, off : off + ncols] = arr.astype(BF16NP)
    biasd = np.zeros((128, 4), np.float32)
    biasd[:, 0] = b1[0:128]
    biasd[:, 1] = b1[128:256]
    biasd[:, 2] = b2
    biasd[:, 3] = bo + dense_b[0] * wo[0]
    return dict(
        padbf=padbf,
        wpack=wpack,
        biasd=biasd,
        identd=np.eye(128, dtype=BF16NP),
    )


def make_core_inputs(shared, cf, nflat, rlo, nblk):
    r_rows = nblk * BLK_ROWS
    # idx16[:, f, b, :]: idx j (= local row c*128+p of block b) wrapped at
    # [j % 16, j // 16], replicated across the 8 gpsimd cores (16-row tiles).
    cb = cf[rlo : rlo + r_rows].reshape(nblk, BLK_ROWS, F)  # [b, j, f]
    w16 = cb.transpose(2, 0, 1).reshape(F, nblk, 128, 16).transpose(0, 1, 3, 2)
    idxk = np.tile(w16, (1, 1, 8, 1)).astype(np.int16)  # [F, nblk, 128, 128]
    idxk = np.ascontiguousarray(idxk.transpose(2, 0, 1, 3))  # [128, F, nblk, 128]
    nTk = np.ascontiguousarray(nflat[rlo : rlo + r_rows].T.astype(BF16NP))
    m = dict(shared)
    m["idx16"] = idxk
    m["idx0"] = np.ascontiguousarray(idxk[:, :, 0, :])
    m["nT"] = nTk
    return m


_NC_CACHE = {}
_RUN_KWARGS = {}


def kernel(**inputs):
    n_features = np.asarray(inputs["n_features"], np.float32)
    c_features = np.asarray(inputs["c_features"])

    shared = prep_weights(inputs)
    cf = c_features.reshape(ROWS, F).astype(np.int32)
    nflat = n_features.reshape(ROWS, FN)

    nblk = R // BLK_ROWS
    if nblk not in _NC_CACHE:
        _NC_CACHE[nblk] = build_program(nblk)
    nc = _NC_CACHE[nblk]

    in_maps = [
        make_core_inputs(shared, cf, nflat, k * R, nblk) for k in range(NCORES)
    ]
    kres = run_bass_kernel_spmd(nc, in_maps, list(range(NCORES)), **_RUN_KWARGS)
    kernel.last_results = kres
    res = kres.results

    out = np.empty((ROWS, OUT), np.float32)
    for k in range(NCORES):
        out[k * R : (k + 1) * R] = res[k]["outT"].T
    return out.reshape(B, G, OUT)


if __name__ == "__main__":
    print("building program...")
    nc = build_program(R // BLK_ROWS)
    print("ok")


# revision 16
# speedup vs baseline: 1.0052x; 1.0052x over previous
"""DeepFM (embedding_lookup) Trainium2 kernel.

Pure data parallel over the flattened (B*G) row dimension across 8
NeuronCores. Per core (16K rows, 8 blocks of 2048):

- Embedding lookup via gpsimd dma_gather (mlp ucode), one call per
  (field, block): 2048 int16 indices gather 256B rows (128 bf16:
  [emb2d(16) | emb1d(1) | sum(emb2d^2)(1) | pad]) from a per-field DRAM
  table. Descgen runs on the Q7 pair (2q, 2q+1) selected by queue_num at
  ~8ns/idx/pair (probe-verified; invariant to call size and elem size), so
  queue_num = field%4 works all four pairs. Sustaining that requires:
  (a) the Pool queue holds ONLY gathers - idxs preloaded via nc.sync (a
      small contiguous idx0 input first so block 0 never waits the 4MB
      bulk load; strided idx DMAs spray thousands of descriptors that
      steal SDMA slots from the gathers - keep them contiguous),
      num_idxs hoisted to one register, all other DMAs on HWDGE;
  (b) two DMASW sem lanes per queue (q and q+4, alternating) - Tile
      serializes reuse of a lane, so one lane per queue caps each queue at
      one call in flight and adds the ~4us completion tail to every call;
  (c) the DVE queue holds ONLY the gather compaction copies - PSUM->SBUF
      xT copies ride ScalarE(Copy), else compacts queue behind the prior
      block's DNN copies (in-order DVE) and stall gathers ~20us per block.
  Steady state: 16.9us per 4-call group = the pure descgen floor.
- DVE compacts the 18 useful bf16 of each row into [128, chunk, field, 18].
- The DNN runs transposed (features on partitions, rows on free dim) in
  bf16 (4x PE rate vs fp32r): PE transposes each 256-row tile (DVE
  StreamTranspose is per-32x32-block only - can't replace it), fp32 PSUM
  accumulate, PSUM->SBUF copies fuse bias+relu on ScalarE.

FM algebra folded into the output matmul:
  out = relu(pe @ wo + bo),  pe = [fm1, fm2, h2]
  fm1 = sum_f e1d_f + n @ dense_w + dense_b      -> wo row 0 contributions:
        e1d columns of xT get weight wo[0]; (dense_w @ wo[0:1]) rides the nT
        K-chunk; dense_b*wo[0] folds into the output bias.
  fm2 = 0.5*(sum_d s1_d^2 - sum_f sq_f), s1 = sum_f e_f, sq_f = sum_d e_fd^2
        -> sq columns of xT get weight -0.5*wo[1]; s1 comes from a small
        selection matmul; s1^2 enters via lhsT = tile(0.5*wo[1]).

Measured (NTFF, 8 cores): 626us, rel err 4.3e-3 (gate 2e-2). Structure:
~21us head (mlp library IRAM load - floor) + 542us gather span (128 calls
x 16.9us/4 - the Q7 descgen floor) + ~65us tail (last blocks' DNN; PE is
~53us busy/block, likely HAM 4/8 clock throttle from bursty duty - tile-
stage pipelining and last-block splitting both measured neutral/negative).
The next lever would be the 8ns/idx ucode descgen itself (no Xtensa
toolchain in-container to rebuild).
"""

import sys

sys.path.insert(0, "/opt/trn_rl_repo")

from contextlib import ExitStack

import numpy as np
import ml_dtypes

import concourse.tile as tile
from concourse import bacc, bass, mybir
from concourse.bass_utils import run_bass_kernel_spmd
from concourse import library_config

BF16NP = np.dtype(ml_dtypes.bfloat16)

B, G = 128, 1024
F, V, D = 16, 10000, 16
FN = 13
H1, H2, OUT = 256, 128, 128
NCORES = 8
ROWS = B * G
R = ROWS // NCORES  # rows per core
AUGW = 18  # e(16) | e1d | sumsq
PADW = 128  # padded DRAM row in bf16 elems (256B, dma_gather elem quantum)
XW = F * AUGW  # 288 transposed feature rows

BLK_ROWS = 2048
NCHUNK = BLK_ROWS // 128  # 16 chunks of 128 rows per block
TPB = NCHUNK // 2  # 8 tiles of 256 rows per block

F32 = mybir.dt.float32
BF16 = mybir.dt.bfloat16
I16 = mybir.dt.int16
RELU = mybir.ActivationFunctionType.Relu
SQUARE = mybir.ActivationFunctionType.Square
COPYF = mybir.ActivationFunctionType.Copy

# Packed weights layout: name -> (col offset, n cols, n valid rows)
WOFFS = {
    "w1c0": (0, 256, 128), "w1c1": (256, 256, 128), "w1c2": (512, 256, 45),
    "sc0": (768, 16, 128), "sc1": (784, 16, 128), "sc2": (800, 16, 32),
    "w2c0": (816, 128, 128), "w2c1": (944, 128, 128),
    "woc0": (1072, 128, 128), "woc1": (1200, 128, 128), "woc2": (1328, 128, 45),
    "wsq": (1456, 128, 16), "wo2": (1584, 128, 128),
}
WPACK_COLS = 1712


def _pin_swdge_sems_to_queues():
    """Pin each Pool DMA's DMASW sem lane to queue_num + 4*(per-queue count%2):
    lanes alternate within a queue so Tile's serialize-on-lane-reuse allows TWO
    calls in flight per queue (descgen of call k+4 overlaps the drain/completion
    tail of call k). Lane mod 4 == queue keeps per-lane completion order = queue
    FIFO order, which the ucode's ring reclaim requires."""
    import concourse.tile_sem_assignment as tsa

    if getattr(tsa.TileClockTick, "_qpinned", False):
        return
    orig = tsa.TileClockTick._assign_tick
    counts = {}

    def patched(self, inst):
        if isinstance(inst, tsa.DMAInst) and inst.engine == mybir.EngineType.Pool:
            q = getattr(inst, "queue_num", 0) or 0
            c = counts.get(q, 0)
            counts[q] = c + 1
            self.next_sw_dma_idx = q + 4 * (c % 2)
        return orig(self, inst)

    tsa.TileClockTick._assign_tick = patched
    tsa.TileClockTick._qpinned = True


def build_program(nblk):
    _pin_swdge_sems_to_queues()
    nc = bacc.Bacc(
        "TRN2",
        target_bir_lowering=False,
        debug=False,
        num_swdge_queues=4,
        dynamic_dma_scratch_size=32768,
    )

    def din(name, shape, dt):
        return nc.dram_tensor(name, list(shape), dt, kind="ExternalInput").ap()

    r_rows = nblk * BLK_ROWS
    padbf = din("padbf", [F, V, PADW], BF16)
    idx16 = din("idx16", [128, F, nblk, 128], I16)
    idx0 = din("idx0", [128, F, 128], I16)
    nT = din("nT", [FN, r_rows], BF16)
    wpack = din("wpack", [128, WPACK_COLS], BF16)
    biasd = din("biasd", [128, 4], F32)
    identd = din("identd", [128, 128], BF16)
    outT = nc.dram_tensor("outT", [128, r_rows], F32, kind="ExternalOutput").ap()

    with tile.TileContext(nc) as tc, ExitStack() as ctx:
        singles = ctx.enter_context(tc.tile_pool(name="singles", bufs=1))
        gfp = ctx.enter_context(tc.tile_pool(name="gfp", bufs=10))
        gp = ctx.enter_context(tc.tile_pool(name="gather", bufs=2))
        op = ctx.enter_context(tc.tile_pool(name="outbuf", bufs=2))
        xs = ctx.enter_context(tc.tile_pool(name="xs", bufs=4))
        hs = ctx.enter_context(tc.tile_pool(name="hs", bufs=3))
        ps_xt = ctx.enter_context(tc.tile_pool(name="ps_xt", bufs=2, space="PSUM"))
        ps_h1 = ctx.enter_context(tc.tile_pool(name="ps_h1", bufs=2, space="PSUM"))
        ps_h2 = ctx.enter_context(tc.tile_pool(name="ps_h2", bufs=1, space="PSUM"))
        ps_wo = ctx.enter_context(tc.tile_pool(name="ps_wo", bufs=2, space="PSUM"))
        ps_s1 = ctx.enter_context(tc.tile_pool(name="ps_s1", bufs=1, space="PSUM"))

        nc.gpsimd.load_library(library_config.mlp)

        idx0s = singles.tile([128, F, 128], I16)
        nc.sync.dma_start(out=idx0s[:], in_=idx0[:])
        ident = singles.tile([128, 128], BF16)
        nc.sync.dma_start(out=ident[:], in_=identd[:])
        idxs = singles.tile([128, F, nblk, 128], I16)
        nc.sync.dma_start(out=idxs[:], in_=idx16[:])
        wpk = singles.tile([128, WPACK_COLS], BF16)
        nc.sync.dma_start(out=wpk[:], in_=wpack[:])
        bias = singles.tile([128, 4], F32)
        nc.sync.dma_start(out=bias[:], in_=biasd[:])
        nTs = singles.tile([FN, r_rows], BF16)
        nc.sync.dma_start(out=nTs[:], in_=nT[:])

        wt = {
            name: wpk[0:nrows, off : off + ncols]
            for name, (off, ncols, nrows) in WOFFS.items()
        }

        # Priming transpose: absorbs the ident-DMA dependency into PE program
        # order so later transposes carry only the gather wait (the S3_LW
        # slot fits a single sync wait).
        warm = ps_wo.tile([128, 128], BF16, space="PSUM", tag="pwo")
        nc.tensor.transpose(out=warm[:], in_=ident[:], identity=ident[:])
        nc.tensor.transpose(out=warm[:], in_=wpk[:, 0:128], identity=ident[:])
        scr = singles.tile([128, 1], F32, tag="scr")
        nc.scalar.activation(out=scr[:], in_=bias[:, 0:1], func=mybir.ActivationFunctionType.Copy)
        scr2 = singles.tile([FN, 1], BF16, tag="scr2")
        nc.vector.tensor_copy(out=scr2[:], in_=nTs[:, 0:1])

        nreg = nc.gpsimd.to_reg(BLK_ROWS)

        for b in range(nblk):
            blk = gp.tile([128, NCHUNK, F, AUGW], BF16)
            for f in range(F):
                gf = gfp.tile([128, NCHUNK, PADW], BF16, tag="gf")
                nc.gpsimd.dma_gather(
                    out_ap=gf[:],
                    in_ap=padbf[f],
                    idxs_ap=idx0s[:, f, :] if b == 0 else idxs[:, f, b, :],
                    num_idxs=BLK_ROWS,
                    num_idxs_reg=nreg,
                    elem_size=PADW,
                    queue_num=f % 4,
                    single_packet=False,
                )
                nc.vector.tensor_copy(out=blk[:, :, f, :], in_=gf[:, :, 0:AUGW])
            blkf = blk[:].rearrange("p c f j -> p c (f j)")  # [128, NCHUNK, 288]
            outb = op.tile([128, BLK_ROWS], F32)

            for t in range(TPB):
                rlo = b * BLK_ROWS + t * 256
                # ---- transpose x tile [256 rows, 288 cols] -> xT pieces ----
                pxt = ps_xt.tile([128, 768], BF16, space="PSUM")
                for ci in range(2):
                    ch = 2 * t + ci
                    for k in range(3):
                        lo = k * 128
                        hi = min(XW, lo + 128)
                        if k < 2:
                            dst = pxt[:, k * 256 + ci * 128 : k * 256 + (ci + 1) * 128]
                        else:
                            dst = pxt[0:32, 512 + ci * 128 : 512 + (ci + 1) * 128]
                        nc.tensor.transpose(
                            out=dst, in_=blkf[:, ch, lo:hi], identity=ident[:]
                        )
                xt0 = xs.tile([128, 256], BF16)
                xt1 = xs.tile([128, 256], BF16)
                xt2 = xs.tile([45, 256], BF16)
                nc.scalar.activation(out=xt0[:], in_=pxt[:, 0:256], func=COPYF)
                nc.scalar.activation(out=xt1[:], in_=pxt[:, 256:512], func=COPYF)
                nc.scalar.activation(out=xt2[0:32, :], in_=pxt[0:32, 512:768], func=COPYF)
                nc.scalar.activation(out=xt2[32:45, :], in_=nTs[:, rlo : rlo + 256], func=COPYF)

                # ---- s1 = sum_f e_f  (transposed: [16, 256]) ----
                ps1 = ps_s1.tile([16, 256], F32, space="PSUM")
                nc.tensor.matmul(ps1[:], wt["sc0"][:], xt0[:], start=True, stop=False)
                nc.tensor.matmul(ps1[:], wt["sc1"][:], xt1[:], start=False, stop=False)
                nc.tensor.matmul(ps1[:], wt["sc2"][:], xt2[0:32, :], start=False, stop=True)
                s1sq = xs.tile([16, 256], BF16)
                nc.scalar.activation(out=s1sq[:], in_=ps1[:], func=SQUARE)

                # ---- h1 = relu(x @ w1 + b1)  -> h1T [256, rows] in 2 halves ----
                ph1 = ps_h1.tile([128, 512], F32, space="PSUM")
                for mh in range(2):
                    dst = ph1[:, mh * 256 : (mh + 1) * 256]
                    ms = slice(mh * 128, (mh + 1) * 128)
                    nc.tensor.matmul(dst, wt["w1c0"][:, ms], xt0[:], start=True, stop=False)
                    nc.tensor.matmul(dst, wt["w1c1"][:, ms], xt1[:], start=False, stop=False)
                    nc.tensor.matmul(dst, wt["w1c2"][:, ms], xt2[:], start=False, stop=True)
                h1t = hs.tile([128, 512], BF16)
                nc.scalar.activation(out=h1t[:, 0:256], in_=ph1[:, 0:256], func=RELU, bias=bias[:, 0:1])
                nc.scalar.activation(out=h1t[:, 256:512], in_=ph1[:, 256:512], func=RELU, bias=bias[:, 1:2])

                # ---- h2 = relu(h1 @ w2 + b2) -> h2T [128, rows] ----
                ph2 = ps_h2.tile([128, 256], F32, space="PSUM")
                nc.tensor.matmul(ph2[:], wt["w2c0"][:], h1t[:, 0:256], start=True, stop=False)
                nc.tensor.matmul(ph2[:], wt["w2c1"][:], h1t[:, 256:512], start=False, stop=True)
                h2t = hs.tile([128, 256], BF16)
                nc.scalar.activation(out=h2t[:], in_=ph2[:], func=RELU, bias=bias[:, 2:3])

                # ---- out = relu(pe @ wo + bo') -> outT [128, rows] ----
                pwo = ps_wo.tile([128, 256], F32, space="PSUM")
                nc.tensor.matmul(pwo[:], wt["woc0"][:], xt0[:], start=True, stop=False)
                nc.tensor.matmul(pwo[:], wt["woc1"][:], xt1[:], start=False, stop=False)
                nc.tensor.matmul(pwo[:], wt["woc2"][:], xt2[:], start=False, stop=False)
                nc.tensor.matmul(pwo[:], wt["wsq"][:], s1sq[:], start=False, stop=False)
                nc.tensor.matmul(pwo[:], wt["wo2"][:], h2t[:], start=False, stop=True)
                nc.scalar.activation(
                    out=outb[:, t * 256 : (t + 1) * 256], in_=pwo[:], func=RELU, bias=bias[:, 3:4]
                )

            nc.sync.dma_start(
                out=outT[:, b * BLK_ROWS : (b + 1) * BLK_ROWS], in_=outb[:]
            )
            if b < nblk - 2:
                # keep the PE's HAM clock at 8/8 through the inter-block idle
                # gap: ~6-12us of dependency-free transposes that drain before
                # the next block's DNN is ready (PE is in-order, and the DNN
                # has ~15us of slack per block to absorb any overrun)
                for _ in range(36):
                    nc.tensor.transpose(out=warm[:], in_=ident[:], identity=ident[:])

    nc.compile()
    return nc


def prep_weights(inputs):
    emb1d = np.asarray(inputs["emb1d"], np.float32)
    emb2d = np.asarray(inputs["emb2d"], np.float32)
    dense_w = np.asarray(inputs["dense_w"], np.float32)
    dense_b = np.asarray(inputs["dense_b"], np.float32)
    w1 = np.asarray(inputs["w1"], np.float32)
    b1 = np.asarray(inputs["b1"], np.float32)
    w2 = np.asarray(inputs["w2"], np.float32)
    b2 = np.asarray(inputs["b2"], np.float32)
    wo = np.asarray(inputs["wo"], np.float32)
    bo = np.asarray(inputs["bo"], np.float32)

    padbf = np.zeros((F, V, PADW), BF16NP)
    padbf[:, :, 0:D] = emb2d.astype(BF16NP)
    padbf[:, :, D] = emb1d[:, :, 0].astype(BF16NP)
    padbf[:, :, D + 1] = (emb2d * emb2d).sum(2).astype(BF16NP)

    # x columns (f-major e layout) -> augmented-row column index
    cols = np.arange(F * D)
    fidx = (cols // D) * AUGW + (cols % D)
    w1aug = np.zeros((XW, H1), np.float32)
    w1aug[fidx] = w1[0 : F * D]
    w1c2 = np.zeros((45, H1), np.float32)
    w1c2[0:32] = w1aug[256:288]
    w1c2[32:45] = w1[F * D : F * D + FN]

    saug = np.zeros((XW, D), np.float32)
    saug[fidx, cols % D] = 1.0

    woaug = np.zeros((XW, OUT), np.float32)
    woaug[np.arange(F) * AUGW + D] = wo[0]  # e1d cols -> fm1_sparse * wo0
    woaug[np.arange(F) * AUGW + D + 1] = -0.5 * wo[1]  # sq cols -> -0.5*t2*wo1
    woc2 = np.zeros((45, OUT), np.float32)
    woc2[0:32] = woaug[256:288]
    woc2[32:45] = dense_w.reshape(FN, 1) @ wo[0:1]

    blocks = dict(
        w1c0=w1aug[0:128], w1c1=w1aug[128:256], w1c2=w1c2,
        sc0=saug[0:128], sc1=saug[128:256], sc2=saug[256:288],
        w2c0=w2[0:128], w2c1=w2[128:256],
        woc0=woaug[0:128], woc1=woaug[128:256], woc2=woc2,
        wsq=np.tile(0.5 * wo[1:2], (D, 1)),
        wo2=wo[2:130],
    )
    wpack = np.zeros((128, WPACK_COLS), BF16NP)
    for name, (off, ncols, nrows) in WOFFS.items():
        arr = blocks[name]
        assert arr.shape == (nrows, ncols), (name, arr.shape, (nrows, ncols))
        wpack[0:nrows, off : off + ncols] = arr.astype(BF16NP)
    biasd = np.zeros((128, 4), np.float32)
    biasd[:, 0] = b1[0:128]
    biasd[:, 1] = b1[128:256]
    biasd[:, 2] = b2
    biasd[:, 3] = bo + dense_b[0] * wo[0]
    return dict(
        padbf=padbf,
        wpack=wpack,
        biasd=biasd,
        identd=np.eye(128, dtype=BF16NP),
    )


def make_core_inputs(shared, cf, nflat, rlo, nblk):
    r_rows = nblk * BLK_ROWS
    # idx16[:, f, b, :]: idx j (= local row c*128+p of block b) wrapped at
    # [j % 16, j // 16], replicated across the 8 gpsimd cores (16-row tiles).
    cb = cf[rlo : rlo + r_rows].reshape(nblk, BLK_ROWS, F)  # [b, j, f]
    w16 = cb.transpose(2, 0, 1).reshape(F, nblk, 128, 16).transpose(0, 1, 3, 2)
    idxk = np.tile(w16, (1, 1, 8, 1)).astype(np.int16)  # [F, nblk, 128, 128]
    idxk = np.ascontiguousarray(idxk.transpose(2, 0, 1, 3))  # [128, F, nblk, 128]
    nTk = np.ascontiguousarray(nflat[rlo : rlo + r_rows].T.astype(BF16NP))
    m = dict(shared)
    m["idx16"] = idxk
    m["idx0"] = np.ascontiguousarray(idxk[:, :, 0, :])
    m["nT"] = nTk
    return m


_NC_CACHE = {}
_RUN_KWARGS = {}


def kernel(**inputs):
    n_features = np.asarray(inputs["n_features"], np.float32)
    c_features = np.asarray(inputs["c_features"])

    shared = prep_weights(inputs)
    cf = c_features.reshape(ROWS, F).astype(np.int32)
    nflat = n_features.reshape(ROWS, FN)

    nblk = R // BLK_ROWS
    if nblk not in _NC_CACHE:
        _NC_CACHE[nblk] = build_program(nblk)
    nc = _NC_CACHE[nblk]

    in_maps = [
        make_core_inputs(shared, cf, nflat, k * R, nblk) for k in range(NCORES)
    ]
    kres = run_bass_kernel_spmd(nc, in_maps, list(range(NCORES)), **_RUN_KWARGS)
    kernel.last_results = kres
    res = kres.results

    out = np.empty((ROWS, OUT), np.float32)
    for k in range(NCORES):
        out[k * R : (k + 1) * R] = res[k]["outT"].T
    return out.reshape(B, G, OUT)


if __name__ == "__main__":
    print("building program...")
    nc = build_program(R // BLK_ROWS)
    print("ok")


# revision 17
# speedup vs baseline: 1.0100x; 1.0048x over previous
"""DeepFM (embedding_lookup) Trainium2 kernel.

Pure data parallel over the flattened (B*G) row dimension across 8
NeuronCores. Per core (16K rows, 8 blocks of 2048):

- Embedding lookup via gpsimd dma_gather (mlp ucode), one call per
  (field, block): 2048 int16 indices gather 256B rows (128 bf16:
  [emb2d(16) | emb1d(1) | sum(emb2d^2)(1) | pad]) from a per-field DRAM
  table. Descgen runs on the Q7 pair (2q, 2q+1) selected by queue_num at
  ~8ns/idx/pair (probe-verified; invariant to call size and elem size), so
  queue_num = field%4 works all four pairs. Sustaining that requires:
  (a) the Pool queue holds ONLY gathers - idxs preloaded via nc.sync (a
      small contiguous idx0 input first so block 0 never waits the 4MB
      bulk load; strided idx DMAs spray thousands of descriptors that
      steal SDMA slots from the gathers - keep them contiguous),
      num_idxs hoisted to one register, all other DMAs on HWDGE;
  (b) two DMASW sem lanes per queue (q and q+4, alternating) - Tile
      serializes reuse of a lane, so one lane per queue caps each queue at
      one call in flight and adds the ~4us completion tail to every call;
  (c) the DVE queue holds ONLY the gather compaction copies - PSUM->SBUF
      xT copies ride ScalarE(Copy), else compacts queue behind the prior
      block's DNN copies (in-order DVE) and stall gathers ~20us per block.
  Steady state: 16.9us per 4-call group = the pure descgen floor.
- DVE compacts the 18 useful bf16 of each row into [128, chunk, field, 18].
- The DNN runs transposed (features on partitions, rows on free dim) in
  bf16 (4x PE rate vs fp32r): PE transposes each 256-row tile (DVE
  StreamTranspose is per-32x32-block only - can't replace it), fp32 PSUM
  accumulate, PSUM->SBUF copies fuse bias+relu on ScalarE.

FM algebra folded into the output matmul:
  out = relu(pe @ wo + bo),  pe = [fm1, fm2, h2]
  fm1 = sum_f e1d_f + n @ dense_w + dense_b      -> wo row 0 contributions:
        e1d columns of xT get weight wo[0]; (dense_w @ wo[0:1]) rides the nT
        K-chunk; dense_b*wo[0] folds into the output bias.
  fm2 = 0.5*(sum_d s1_d^2 - sum_f sq_f), s1 = sum_f e_f, sq_f = sum_d e_fd^2
        -> sq columns of xT get weight -0.5*wo[1]; s1 comes from a small
        selection matmul; s1^2 enters via lhsT = tile(0.5*wo[1]).

Measured (NTFF, 8 cores): 626us, rel err 4.3e-3 (gate 2e-2). Structure:
~21us head (mlp library IRAM load - floor) + 542us gather span (128 calls
x 16.9us/4 - the Q7 descgen floor) + ~65us tail (last blocks' DNN; PE is
~53us busy/block, likely HAM 4/8 clock throttle from bursty duty - tile-
stage pipelining and last-block splitting both measured neutral/negative).
The next lever would be the 8ns/idx ucode descgen itself (no Xtensa
toolchain in-container to rebuild).
"""

import sys

sys.path.insert(0, "/opt/trn_rl_repo")

from contextlib import ExitStack

import numpy as np
import ml_dtypes

import concourse.tile as tile
from concourse import bacc, bass, mybir
from concourse.bass_utils import run_bass_kernel_spmd
from concourse import library_config

BF16NP = np.dtype(ml_dtypes.bfloat16)

B, G = 128, 1024
F, V, D = 16, 10000, 16
FN = 13
H1, H2, OUT = 256, 128, 128
NCORES = 8
ROWS = B * G
R = ROWS // NCORES  # rows per core
AUGW = 18  # e(16) | e1d | sumsq
PADW = 128  # padded DRAM row in bf16 elems (256B, dma_gather elem quantum)
XW = F * AUGW  # 288 transposed feature rows

BLK_ROWS = 2048
NCHUNK = BLK_ROWS // 128  # 16 chunks of 128 rows per block
TPB = NCHUNK // 2  # 8 tiles of 256 rows per block

F32 = mybir.dt.float32
BF16 = mybir.dt.bfloat16
I16 = mybir.dt.int16
RELU = mybir.ActivationFunctionType.Relu
SQUARE = mybir.ActivationFunctionType.Square
COPYF = mybir.ActivationFunctionType.Copy

# Packed weights layout: name -> (col offset, n cols, n valid rows)
WOFFS = {
    "w1c0": (0, 256, 128), "w1c1": (256, 256, 128), "w1c2": (512, 256, 45),
    "sc0": (768, 16, 128), "sc1": (784, 16, 128), "sc2": (800, 16, 32),
    "w2c0": (816, 128, 128), "w2c1": (944, 128, 128),
    "woc0": (1072, 128, 128), "woc1": (1200, 128, 128), "woc2": (1328, 128, 45),
    "wsq": (1456, 128, 16), "wo2": (1584, 128, 128),
}
WPACK_COLS = 1712


def _pin_swdge_sems_to_queues():
    """Pin each Pool DMA's DMASW sem lane to queue_num + 4*(per-queue count%2):
    lanes alternate within a queue so Tile's serialize-on-lane-reuse allows TWO
    calls in flight per queue (descgen of call k+4 overlaps the drain/completion
    tail of call k). Lane mod 4 == queue keeps per-lane completion order = queue
    FIFO order, which the ucode's ring reclaim requires."""
    import concourse.tile_sem_assignment as tsa

    if getattr(tsa.TileClockTick, "_qpinned", False):
        return
    orig = tsa.TileClockTick._assign_tick
    counts = {}

    def patched(self, inst):
        if isinstance(inst, tsa.DMAInst) and inst.engine == mybir.EngineType.Pool:
            q = getattr(inst, "queue_num", 0) or 0
            c = counts.get(q, 0)
            counts[q] = c + 1
            self.next_sw_dma_idx = q + 4 * (c % 2)
        return orig(self, inst)

    tsa.TileClockTick._assign_tick = patched
    tsa.TileClockTick._qpinned = True


def build_program(nblk):
    _pin_swdge_sems_to_queues()
    nc = bacc.Bacc(
        "TRN2",
        target_bir_lowering=False,
        debug=False,
        num_swdge_queues=4,
        dynamic_dma_scratch_size=32768,
    )

    def din(name, shape, dt):
        return nc.dram_tensor(name, list(shape), dt, kind="ExternalInput").ap()

    r_rows = nblk * BLK_ROWS
    padbf = din("padbf", [F, V, PADW], BF16)
    idx16 = din("idx16", [128, F, nblk, 128], I16)
    idx0 = din("idx0", [128, F, 128], I16)
    nT = din("nT", [FN, r_rows], BF16)
    wpack = din("wpack", [128, WPACK_COLS], BF16)
    biasd = din("biasd", [128, 4], F32)
    identd = din("identd", [128, 128], BF16)
    outT = nc.dram_tensor("outT", [128, r_rows], F32, kind="ExternalOutput").ap()

    with tile.TileContext(nc) as tc, ExitStack() as ctx:
        singles = ctx.enter_context(tc.tile_pool(name="singles", bufs=1))
        gfp = ctx.enter_context(tc.tile_pool(name="gfp", bufs=10))
        gp = ctx.enter_context(tc.tile_pool(name="gather", bufs=2))
        op = ctx.enter_context(tc.tile_pool(name="outbuf", bufs=2))
        xs = ctx.enter_context(tc.tile_pool(name="xs", bufs=4))
        hs = ctx.enter_context(tc.tile_pool(name="hs", bufs=3))
        ps_xt = ctx.enter_context(tc.tile_pool(name="ps_xt", bufs=2, space="PSUM"))
        ps_h1 = ctx.enter_context(tc.tile_pool(name="ps_h1", bufs=2, space="PSUM"))
        ps_h2 = ctx.enter_context(tc.tile_pool(name="ps_h2", bufs=1, space="PSUM"))
        ps_wo = ctx.enter_context(tc.tile_pool(name="ps_wo", bufs=2, space="PSUM"))
        ps_s1 = ctx.enter_context(tc.tile_pool(name="ps_s1", bufs=1, space="PSUM"))

        nc.gpsimd.load_library(library_config.mlp)

        idx0s = singles.tile([128, F, 128], I16)
        nc.sync.dma_start(out=idx0s[:], in_=idx0[:])
        ident = singles.tile([128, 128], BF16)
        nc.sync.dma_start(out=ident[:], in_=identd[:])
        idxs = singles.tile([128, F, nblk, 128], I16)
        nc.sync.dma_start(out=idxs[:], in_=idx16[:])
        wpk = singles.tile([128, WPACK_COLS], BF16)
        nc.sync.dma_start(out=wpk[:], in_=wpack[:])
        bias = singles.tile([128, 4], F32)
        nc.sync.dma_start(out=bias[:], in_=biasd[:])
        nTs = singles.tile([FN, r_rows], BF16)
        nc.sync.dma_start(out=nTs[:], in_=nT[:])

        wt = {
            name: wpk[0:nrows, off : off + ncols]
            for name, (off, ncols, nrows) in WOFFS.items()
        }

        # Priming transpose: absorbs the ident-DMA dependency into PE program
        # order so later transposes carry only the gather wait (the S3_LW
        # slot fits a single sync wait).
        warm = ps_wo.tile([128, 128], BF16, space="PSUM", tag="pwo")
        nc.tensor.transpose(out=warm[:], in_=ident[:], identity=ident[:])
        nc.tensor.transpose(out=warm[:], in_=wpk[:, 0:128], identity=ident[:])
        scr = singles.tile([128, 1], F32, tag="scr")
        nc.scalar.activation(out=scr[:], in_=bias[:, 0:1], func=mybir.ActivationFunctionType.Copy)
        scr2 = singles.tile([FN, 1], BF16, tag="scr2")
        nc.vector.tensor_copy(out=scr2[:], in_=nTs[:, 0:1])

        nreg = nc.gpsimd.to_reg(BLK_ROWS)

        for b in range(nblk):
            blk = gp.tile([128, NCHUNK, F, AUGW], BF16)
            for f in range(F):
                gf = gfp.tile([128, NCHUNK, PADW], BF16, tag="gf")
                nc.gpsimd.dma_gather(
                    out_ap=gf[:],
                    in_ap=padbf[f],
                    idxs_ap=idx0s[:, f, :] if b == 0 else idxs[:, f, b, :],
                    num_idxs=BLK_ROWS,
                    num_idxs_reg=nreg,
                    elem_size=PADW,
                    queue_num=f % 4,
                    single_packet=False,
                )
                nc.vector.tensor_copy(out=blk[:, :, f, :], in_=gf[:, :, 0:AUGW])
            blkf = blk[:].rearrange("p c f j -> p c (f j)")  # [128, NCHUNK, 288]
            outb = op.tile([128, BLK_ROWS], F32)

            for t in range(TPB):
                rlo = b * BLK_ROWS + t * 256
                # ---- transpose x tile [256 rows, 288 cols] -> xT pieces ----
                pxt = ps_xt.tile([128, 768], BF16, space="PSUM")
                for ci in range(2):
                    ch = 2 * t + ci
                    for k in range(3):
                        lo = k * 128
                        hi = min(XW, lo + 128)
                        if k < 2:
                            dst = pxt[:, k * 256 + ci * 128 : k * 256 + (ci + 1) * 128]
                        else:
                            dst = pxt[0:32, 512 + ci * 128 : 512 + (ci + 1) * 128]
                        nc.tensor.transpose(
                            out=dst, in_=blkf[:, ch, lo:hi], identity=ident[:]
                        )
                xt0 = xs.tile([128, 256], BF16)
                xt1 = xs.tile([128, 256], BF16)
                xt2 = xs.tile([45, 256], BF16)
                nc.scalar.activation(out=xt0[:], in_=pxt[:, 0:256], func=COPYF)
                nc.scalar.activation(out=xt1[:], in_=pxt[:, 256:512], func=COPYF)
                nc.scalar.activation(out=xt2[0:32, :], in_=pxt[0:32, 512:768], func=COPYF)
                nc.scalar.activation(out=xt2[32:45, :], in_=nTs[:, rlo : rlo + 256], func=COPYF)

                # ---- s1 = sum_f e_f  (transposed: [16, 256]) ----
                ps1 = ps_s1.tile([16, 256], F32, space="PSUM")
                nc.tensor.matmul(ps1[:], wt["sc0"][:], xt0[:], start=True, stop=False)
                nc.tensor.matmul(ps1[:], wt["sc1"][:], xt1[:], start=False, stop=False)
                nc.tensor.matmul(ps1[:], wt["sc2"][:], xt2[0:32, :], start=False, stop=True)
                s1sq = xs.tile([16, 256], BF16)
                nc.scalar.activation(out=s1sq[:], in_=ps1[:], func=SQUARE)

                # ---- h1 = relu(x @ w1 + b1)  -> h1T [256, rows] in 2 halves ----
                ph1 = ps_h1.tile([128, 512], F32, space="PSUM")
                for mh in range(2):
                    dst = ph1[:, mh * 256 : (mh + 1) * 256]
                    ms = slice(mh * 128, (mh + 1) * 128)
                    nc.tensor.matmul(dst, wt["w1c0"][:, ms], xt0[:], start=True, stop=False)
                    nc.tensor.matmul(dst, wt["w1c1"][:, ms], xt1[:], start=False, stop=False)
                    nc.tensor.matmul(dst, wt["w1c2"][:, ms], xt2[:], start=False, stop=True)
                h1t = hs.tile([128, 512], BF16)
                nc.scalar.activation(out=h1t[:, 0:256], in_=ph1[:, 0:256], func=RELU, bias=bias[:, 0:1])
                nc.scalar.activation(out=h1t[:, 256:512], in_=ph1[:, 256:512], func=RELU, bias=bias[:, 1:2])

                # ---- h2 = relu(h1 @ w2 + b2) -> h2T [128, rows] ----
                ph2 = ps_h2.tile([128, 256], F32, space="PSUM")
                nc.tensor.matmul(ph2[:], wt["w2c0"][:], h1t[:, 0:256], start=True, stop=False)
                nc.tensor.matmul(ph2[:], wt["w2c1"][:], h1t[:, 256:512], start=False, stop=True)
                h2t = hs.tile([128, 256], BF16)
                nc.scalar.activation(out=h2t[:], in_=ph2[:], func=RELU, bias=bias[:, 2:3])

                # ---- out = relu(pe @ wo + bo') -> outT [128, rows] ----
                pwo = ps_wo.tile([128, 256], F32, space="PSUM")
                nc.tensor.matmul(pwo[:], wt["woc0"][:], xt0[:], start=True, stop=False)
                nc.tensor.matmul(pwo[:], wt["woc1"][:], xt1[:], start=False, stop=False)
                nc.tensor.matmul(pwo[:], wt["woc2"][:], xt2[:], start=False, stop=False)
                nc.tensor.matmul(pwo[:], wt["wsq"][:], s1sq[:], start=False, stop=False)
                nc.tensor.matmul(pwo[:], wt["wo2"][:], h2t[:], start=False, stop=True)
                nc.scalar.activation(
                    out=outb[:, t * 256 : (t + 1) * 256], in_=pwo[:], func=RELU, bias=bias[:, 3:4]
                )

            nc.sync.dma_start(
                out=outT[:, b * BLK_ROWS : (b + 1) * BLK_ROWS], in_=outb[:]
            )

    nc.compile()
    return nc


def prep_weights(inputs):
    emb1d = np.asarray(inputs["emb1d"], np.float32)
    emb2d = np.asarray(inputs["emb2d"], np.float32)
    dense_w = np.asarray(inputs["dense_w"], np.float32)
    dense_b = np.asarray(inputs["dense_b"], np.float32)
    w1 = np.asarray(inputs["w1"], np.float32)
    b1 = np.asarray(inputs["b1"], np.float32)
    w2 = np.asarray(inputs["w2"], np.float32)
    b2 = np.asarray(inputs["b2"], np.float32)
    wo = np.asarray(inputs["wo"], np.float32)
    bo = np.asarray(inputs["bo"], np.float32)

    padbf = np.zeros((F, V, PADW), BF16NP)
    padbf[:, :, 0:D] = emb2d.astype(BF16NP)
    padbf[:, :, D] = emb1d[:, :, 0].astype(BF16NP)
    padbf[:, :, D + 1] = (emb2d * emb2d).sum(2).astype(BF16NP)

    # x columns (f-major e layout) -> augmented-row column index
    cols = np.arange(F * D)
    fidx = (cols // D) * AUGW + (cols % D)
    w1aug = np.zeros((XW, H1), np.float32)
    w1aug[fidx] = w1[0 : F * D]
    w1c2 = np.zeros((45, H1), np.float32)
    w1c2[0:32] = w1aug[256:288]
    w1c2[32:45] = w1[F * D : F * D + FN]

    saug = np.zeros((XW, D), np.float32)
    saug[fidx, cols % D] = 1.0

    woaug = np.zeros((XW, OUT), np.float32)
    woaug[np.arange(F) * AUGW + D] = wo[0]  # e1d cols -> fm1_sparse * wo0
    woaug[np.arange(F) * AUGW + D + 1] = -0.5 * wo[1]  # sq cols -> -0.5*t2*wo1
    woc2 = np.zeros((45, OUT), np.float32)
    woc2[0:32] = woaug[256:288]
    woc2[32:45] = dense_w.reshape(FN, 1) @ wo[0:1]

    blocks = dict(
        w1c0=w1aug[0:128], w1c1=w1aug[128:256], w1c2=w1c2,
        sc0=saug[0:128], sc1=saug[128:256], sc2=saug[256:288],
        w2c0=w2[0:128], w2c1=w2[128:256],
        woc0=woaug[0:128], woc1=woaug[128:256], woc2=woc2,
        wsq=np.tile(0.5 * wo[1:2], (D, 1)),
        wo2=wo[2:130],
    )
    wpack = np.zeros((128, WPACK_COLS), BF16NP)
    for name, (off, ncols, nrows) in WOFFS.items():
        arr = blocks[name]
        assert arr.shape == (nrows, ncols), (name, arr.shape, (nrows, ncols))
        wpack[0:nrows, off : off + ncols] = arr.astype(BF16NP)
    biasd = np.zeros((128, 4), np.float32)
    biasd[:, 0] = b1[0:128]
    biasd[:, 1] = b1[128:256]
    biasd[:, 2] = b2
    biasd[:, 3] = bo + dense_b[0] * wo[0]
    return dict(
        padbf=padbf,
        wpack=wpack,
        biasd=biasd,
        identd=np.eye(128, dtype=BF16NP),
    )


def make_core_inputs(shared, cf, nflat, rlo, nblk):
    r_rows = nblk * BLK_ROWS
    # idx16[:, f, b, :]: idx j (= local row c*128+p of block b) wrapped at
    # [j % 16, j // 16], replicated across the 8 gpsimd cores (16-row tiles).
    cb = cf[rlo : rlo + r_rows].reshape(nblk, BLK_ROWS, F)  # [b, j, f]
    w16 = cb.transpose(2, 0, 1).reshape(F, nblk, 128, 16).transpose(0, 1, 3, 2)
    idxk = np.tile(w16, (1, 1, 8, 1)).astype(np.int16)  # [F, nblk, 128, 128]
    idxk = np.ascontiguousarray(idxk.transpose(2, 0, 1, 3))  # [128, F, nblk, 128]
    nTk = np.ascontiguousarray(nflat[rlo : rlo + r_rows].T.astype(BF16NP))
    m = dict(shared)
    m["idx16"] = idxk
    m["idx0"] = np.ascontiguousarray(idxk[:, :, 0, :])
    m["nT"] = nTk
    return m


_NC_CACHE = {}
_RUN_KWARGS = {}


def kernel(**inputs):
    n_features = np.asarray(inputs["n_features"], np.float32)
    c_features = np.asarray(inputs["c_features"])

    shared = prep_weights(inputs)
    cf = c_features.reshape(ROWS, F).astype(np.int32)
    nflat = n_features.reshape(ROWS, FN)

    nblk = R // BLK_ROWS
    if nblk not in _NC_CACHE:
        _NC_CACHE[nblk] = build_program(nblk)
    nc = _NC_CACHE[nblk]

    in_maps = [
        make_core_inputs(shared, cf, nflat, k * R, nblk) for k in range(NCORES)
    ]
    kres = run_bass_kernel_spmd(nc, in_maps, list(range(NCORES)), **_RUN_KWARGS)
    kernel.last_results = kres
    res = kres.results

    out = np.empty((ROWS, OUT), np.float32)
    for k in range(NCORES):
        out[k * R : (k + 1) * R] = res[k]["outT"].T
    return out.reshape(B, G, OUT)


if __name__ == "__main__":
    print("building program...")
    nc = build_program(R // BLK_ROWS)
    print("ok")


# revision 18
# speedup vs baseline: 1.0196x; 1.0095x over previous
"""DeepFM (embedding_lookup) Trainium2 kernel.

Pure data parallel over the flattened (B*G) row dimension across 8
NeuronCores. Per core (16K rows, 8 blocks of 2048):

- Embedding lookup via gpsimd dma_gather (mlp ucode), one call per
  (field, block): 2048 int16 indices gather 256B rows (128 bf16:
  [emb2d(16) | emb1d(1) | sum(emb2d^2)(1) | pad]) from a per-field DRAM
  table. Descgen runs on the Q7 pair (2q, 2q+1) selected by queue_num at
  ~8ns/idx/pair (probe-verified; invariant to call size and elem size), so
  queue_num = field%4 works all four pairs. Sustaining that requires:
  (a) the Pool queue holds ONLY gathers - idxs preloaded via nc.sync (a
      small contiguous idx0 input first so block 0 never waits the 4MB
      bulk load; strided idx DMAs spray thousands of descriptors that
      steal SDMA slots from the gathers - keep them contiguous),
      num_idxs hoisted to one register, all other DMAs on HWDGE;
  (b) two DMASW sem lanes per queue (q and q+4, alternating) - Tile
      serializes reuse of a lane, so one lane per queue caps each queue at
      one call in flight and adds the ~4us completion tail to every call;
  (c) the DVE queue holds ONLY the gather compaction copies - PSUM->SBUF
      xT copies ride ScalarE(Copy), else compacts queue behind the prior
      block's DNN copies (in-order DVE) and stall gathers ~20us per block.
  Steady state: 16.9us per 4-call group = the pure descgen floor.
- DVE compacts the 18 useful bf16 of each row into [128, chunk, field, 18].
- The DNN runs transposed (features on partitions, rows on free dim) in
  bf16 (4x PE rate vs fp32r): PE transposes each 256-row tile (DVE
  StreamTranspose is per-32x32-block only - can't replace it), fp32 PSUM
  accumulate, PSUM->SBUF copies fuse bias+relu on ScalarE.

FM algebra folded into the output matmul:
  out = relu(pe @ wo + bo),  pe = [fm1, fm2, h2]
  fm1 = sum_f e1d_f + n @ dense_w + dense_b      -> wo row 0 contributions:
        e1d columns of xT get weight wo[0]; (dense_w @ wo[0:1]) rides the nT
        K-chunk; dense_b*wo[0] folds into the output bias.
  fm2 = 0.5*(sum_d s1_d^2 - sum_f sq_f), s1 = sum_f e_f, sq_f = sum_d e_fd^2
        -> sq columns of xT get weight -0.5*wo[1]; s1 comes from a small
        selection matmul; s1^2 enters via lhsT = tile(0.5*wo[1]).

Measured (NTFF, 8 cores): 626us, rel err 4.3e-3 (gate 2e-2). Structure:
~21us head (mlp library IRAM load - floor) + 542us gather span (128 calls
x 16.9us/4 - the Q7 descgen floor) + ~65us tail (last blocks' DNN; PE is
~53us busy/block, likely HAM 4/8 clock throttle from bursty duty - tile-
stage pipelining and last-block splitting both measured neutral/negative).
The next lever would be the 8ns/idx ucode descgen itself (no Xtensa
toolchain in-container to rebuild).
"""

import sys

sys.path.insert(0, "/opt/trn_rl_repo")

from contextlib import ExitStack

import numpy as np
import ml_dtypes

import concourse.tile as tile
from concourse import bacc, bass, mybir
from concourse.bass_utils import run_bass_kernel_spmd
from concourse import library_config

BF16NP = np.dtype(ml_dtypes.bfloat16)

B, G = 128, 1024
F, V, D = 16, 10000, 16
FN = 13
H1, H2, OUT = 256, 128, 128
NCORES = 8
ROWS = B * G
R = ROWS // NCORES  # rows per core
AUGW = 18  # e(16) | e1d | sumsq
PADW = 128  # padded DRAM row in bf16 elems (256B, dma_gather elem quantum)
XW = F * AUGW  # 288 transposed feature rows

BLK_ROWS = 2048
NCHUNK = BLK_ROWS // 128  # 16 chunks of 128 rows per block
TPB = NCHUNK // 2  # 8 tiles of 256 rows per block

F32 = mybir.dt.float32
BF16 = mybir.dt.bfloat16
I16 = mybir.dt.int16
RELU = mybir.ActivationFunctionType.Relu
SQUARE = mybir.ActivationFunctionType.Square
COPYF = mybir.ActivationFunctionType.Copy

# Packed weights layout: name -> (col offset, n cols, n valid rows)
WOFFS = {
    "w1c0": (0, 256, 128), "w1c1": (256, 256, 128), "w1c2": (512, 256, 45),
    "sc0": (768, 16, 128), "sc1": (784, 16, 128), "sc2": (800, 16, 32),
    "w2c0": (816, 128, 128), "w2c1": (944, 128, 128),
    "woc0": (1072, 128, 128), "woc1": (1200, 128, 128), "woc2": (1328, 128, 45),
    "wsq": (1456, 128, 16), "wo2": (1584, 128, 128),
}
WPACK_COLS = 1712


def _pin_swdge_sems_to_queues():
    """Pin each Pool DMA's DMASW sem lane to queue_num + 4*(per-queue count%2):
    lanes alternate within a queue so Tile's serialize-on-lane-reuse allows TWO
    calls in flight per queue (descgen of call k+4 overlaps the drain/completion
    tail of call k). Lane mod 4 == queue keeps per-lane completion order = queue
    FIFO order, which the ucode's ring reclaim requires."""
    import concourse.tile_sem_assignment as tsa

    if getattr(tsa.TileClockTick, "_qpinned", False):
        return
    orig = tsa.TileClockTick._assign_tick
    counts = {}

    def patched(self, inst):
        if isinstance(inst, tsa.DMAInst) and inst.engine == mybir.EngineType.Pool:
            q = getattr(inst, "queue_num", 0) or 0
            c = counts.get(q, 0)
            counts[q] = c + 1
            self.next_sw_dma_idx = q + 4 * (c % 2)
        return orig(self, inst)

    tsa.TileClockTick._assign_tick = patched
    tsa.TileClockTick._qpinned = True


def build_program(nblk):
    _pin_swdge_sems_to_queues()
    nc = bacc.Bacc(
        "TRN2",
        target_bir_lowering=False,
        debug=False,
        num_swdge_queues=4,
        dynamic_dma_scratch_size=32768,
    )

    def din(name, shape, dt):
        return nc.dram_tensor(name, list(shape), dt, kind="ExternalInput").ap()

    r_rows = nblk * BLK_ROWS
    padbf = din("padbf", [F, V, PADW], BF16)
    idx16 = din("idx16", [128, F, nblk, 128], I16)
    idx0 = din("idx0", [128, F, 128], I16)
    nT = din("nT", [FN, r_rows], BF16)
    wpack = din("wpack", [128, WPACK_COLS], BF16)
    biasd = din("biasd", [128, 4], F32)
    identd = din("identd", [128, 128], BF16)
    outT = nc.dram_tensor("outT", [128, r_rows], F32, kind="ExternalOutput").ap()

    with tile.TileContext(nc) as tc, ExitStack() as ctx:
        singles = ctx.enter_context(tc.tile_pool(name="singles", bufs=1))
        gfp = ctx.enter_context(tc.tile_pool(name="gfp", bufs=13))
        gp = ctx.enter_context(tc.tile_pool(name="gather", bufs=3))
        op = ctx.enter_context(tc.tile_pool(name="outbuf", bufs=2))
        xs = ctx.enter_context(tc.tile_pool(name="xs", bufs=4))
        hs = ctx.enter_context(tc.tile_pool(name="hs", bufs=3))
        ps_xt = ctx.enter_context(tc.tile_pool(name="ps_xt", bufs=2, space="PSUM"))
        ps_h1 = ctx.enter_context(tc.tile_pool(name="ps_h1", bufs=2, space="PSUM"))
        ps_h2 = ctx.enter_context(tc.tile_pool(name="ps_h2", bufs=1, space="PSUM"))
        ps_wo = ctx.enter_context(tc.tile_pool(name="ps_wo", bufs=2, space="PSUM"))
        ps_s1 = ctx.enter_context(tc.tile_pool(name="ps_s1", bufs=1, space="PSUM"))

        nc.gpsimd.load_library(library_config.mlp)

        idx0s = singles.tile([128, F, 128], I16)
        nc.sync.dma_start(out=idx0s[:], in_=idx0[:])
        ident = singles.tile([128, 128], BF16)
        nc.sync.dma_start(out=ident[:], in_=identd[:])
        idxs = singles.tile([128, F, nblk, 128], I16)
        nc.sync.dma_start(out=idxs[:], in_=idx16[:])
        wpk = singles.tile([128, WPACK_COLS], BF16)
        nc.sync.dma_start(out=wpk[:], in_=wpack[:])
        bias = singles.tile([128, 4], F32)
        nc.sync.dma_start(out=bias[:], in_=biasd[:])
        nTs = singles.tile([FN, r_rows], BF16)
        nc.sync.dma_start(out=nTs[:], in_=nT[:])

        wt = {
            name: wpk[0:nrows, off : off + ncols]
            for name, (off, ncols, nrows) in WOFFS.items()
        }

        # Priming transpose: absorbs the ident-DMA dependency into PE program
        # order so later transposes carry only the gather wait (the S3_LW
        # slot fits a single sync wait).
        warm = ps_wo.tile([128, 128], BF16, space="PSUM", tag="pwo")
        nc.tensor.transpose(out=warm[:], in_=ident[:], identity=ident[:])
        nc.tensor.transpose(out=warm[:], in_=wpk[:, 0:128], identity=ident[:])
        scr = singles.tile([128, 1], F32, tag="scr")
        nc.scalar.activation(out=scr[:], in_=bias[:, 0:1], func=mybir.ActivationFunctionType.Copy)
        scr2 = singles.tile([FN, 1], BF16, tag="scr2")
        nc.vector.tensor_copy(out=scr2[:], in_=nTs[:, 0:1])

        nreg = nc.gpsimd.to_reg(BLK_ROWS)

        for b in range(nblk):
            blk = gp.tile([128, NCHUNK, F, AUGW], BF16)
            for f in range(F):
                gf = gfp.tile([128, NCHUNK, PADW], BF16, tag="gf")
                nc.gpsimd.dma_gather(
                    out_ap=gf[:],
                    in_ap=padbf[f],
                    idxs_ap=idx0s[:, f, :] if b == 0 else idxs[:, f, b, :],
                    num_idxs=BLK_ROWS,
                    num_idxs_reg=nreg,
                    elem_size=PADW,
                    queue_num=f % 4,
                    single_packet=False,
                )
                nc.vector.tensor_copy(out=blk[:, :, f, :], in_=gf[:, :, 0:AUGW])
            blkf = blk[:].rearrange("p c f j -> p c (f j)")  # [128, NCHUNK, 288]
            outb = op.tile([128, BLK_ROWS], F32)

            for t in range(TPB):
                rlo = b * BLK_ROWS + t * 256
                # ---- transpose x tile [256 rows, 288 cols] -> xT pieces ----
                pxt = ps_xt.tile([128, 768], BF16, space="PSUM")
                for ci in range(2):
                    ch = 2 * t + ci
                    for k in range(3):
                        lo = k * 128
                        hi = min(XW, lo + 128)
                        if k < 2:
                            dst = pxt[:, k * 256 + ci * 128 : k * 256 + (ci + 1) * 128]
                        else:
                            dst = pxt[0:32, 512 + ci * 128 : 512 + (ci + 1) * 128]
                        nc.tensor.transpose(
                            out=dst, in_=blkf[:, ch, lo:hi], identity=ident[:]
                        )
                xt0 = xs.tile([128, 256], BF16)
                xt1 = xs.tile([128, 256], BF16)
                xt2 = xs.tile([45, 256], BF16)
                nc.scalar.activation(out=xt0[:], in_=pxt[:, 0:256], func=COPYF)
                nc.scalar.activation(out=xt1[:], in_=pxt[:, 256:512], func=COPYF)
                nc.scalar.activation(out=xt2[0:32, :], in_=pxt[0:32, 512:768], func=COPYF)
                nc.scalar.activation(out=xt2[32:45, :], in_=nTs[:, rlo : rlo + 256], func=COPYF)

                # ---- s1 = sum_f e_f  (transposed: [16, 256]) ----
                ps1 = ps_s1.tile([16, 256], F32, space="PSUM")
                nc.tensor.matmul(ps1[:], wt["sc0"][:], xt0[:], start=True, stop=False)
                nc.tensor.matmul(ps1[:], wt["sc1"][:], xt1[:], start=False, stop=False)
                nc.tensor.matmul(ps1[:], wt["sc2"][:], xt2[0:32, :], start=False, stop=True)
                s1sq = xs.tile([16, 256], BF16)
                nc.scalar.activation(out=s1sq[:], in_=ps1[:], func=SQUARE)

                # ---- h1 = relu(x @ w1 + b1)  -> h1T [256, rows] in 2 halves ----
                ph1 = ps_h1.tile([128, 512], F32, space="PSUM")
                for mh in range(2):
                    dst = ph1[:, mh * 256 : (mh + 1) * 256]
                    ms = slice(mh * 128, (mh + 1) * 128)
                    nc.tensor.matmul(dst, wt["w1c0"][:, ms], xt0[:], start=True, stop=False)
                    nc.tensor.matmul(dst, wt["w1c1"][:, ms], xt1[:], start=False, stop=False)
                    nc.tensor.matmul(dst, wt["w1c2"][:, ms], xt2[:], start=False, stop=True)
                h1t = hs.tile([128, 512], BF16)
                nc.scalar.activation(out=h1t[:, 0:256], in_=ph1[:, 0:256], func=RELU, bias=bias[:, 0:1])
                nc.scalar.activation(out=h1t[:, 256:512], in_=ph1[:, 256:512], func=RELU, bias=bias[:, 1:2])

                # ---- h2 = relu(h1 @ w2 + b2) -> h2T [128, rows] ----
                ph2 = ps_h2.tile([128, 256], F32, space="PSUM")
                nc.tensor.matmul(ph2[:], wt["w2c0"][:], h1t[:, 0:256], start=True, stop=False)
                nc.tensor.matmul(ph2[:], wt["w2c1"][:], h1t[:, 256:512], start=False, stop=True)
                h2t = hs.tile([128, 256], BF16)
                nc.scalar.activation(out=h2t[:], in_=ph2[:], func=RELU, bias=bias[:, 2:3])

                # ---- out = relu(pe @ wo + bo') -> outT [128, rows] ----
                pwo = ps_wo.tile([128, 256], F32, space="PSUM")
                nc.tensor.matmul(pwo[:], wt["woc0"][:], xt0[:], start=True, stop=False)
                nc.tensor.matmul(pwo[:], wt["woc1"][:], xt1[:], start=False, stop=False)
                nc.tensor.matmul(pwo[:], wt["woc2"][:], xt2[:], start=False, stop=False)
                nc.tensor.matmul(pwo[:], wt["wsq"][:], s1sq[:], start=False, stop=False)
                nc.tensor.matmul(pwo[:], wt["wo2"][:], h2t[:], start=False, stop=True)
                nc.scalar.activation(
                    out=outb[:, t * 256 : (t + 1) * 256], in_=pwo[:], func=RELU, bias=bias[:, 3:4]
                )

            nc.sync.dma_start(
                out=outT[:, b * BLK_ROWS : (b + 1) * BLK_ROWS], in_=outb[:]
            )

    nc.compile()
    return nc


def prep_weights(inputs):
    emb1d = np.asarray(inputs["emb1d"], np.float32)
    emb2d = np.asarray(inputs["emb2d"], np.float32)
    dense_w = np.asarray(inputs["dense_w"], np.float32)
    dense_b = np.asarray(inputs["dense_b"], np.float32)
    w1 = np.asarray(inputs["w1"], np.float32)
    b1 = np.asarray(inputs["b1"], np.float32)
    w2 = np.asarray(inputs["w2"], np.float32)
    b2 = np.asarray(inputs["b2"], np.float32)
    wo = np.asarray(inputs["wo"], np.float32)
    bo = np.asarray(inputs["bo"], np.float32)

    padbf = np.zeros((F, V, PADW), BF16NP)
    padbf[:, :, 0:D] = emb2d.astype(BF16NP)
    padbf[:, :, D] = emb1d[:, :, 0].astype(BF16NP)
    padbf[:, :, D + 1] = (emb2d * emb2d).sum(2).astype(BF16NP)

    # x columns (f-major e layout) -> augmented-row column index
    cols = np.arange(F * D)
    fidx = (cols // D) * AUGW + (cols % D)
    w1aug = np.zeros((XW, H1), np.float32)
    w1aug[fidx] = w1[0 : F * D]
    w1c2 = np.zeros((45, H1), np.float32)
    w1c2[0:32] = w1aug[256:288]
    w1c2[32:45] = w1[F * D : F * D + FN]

    saug = np.zeros((XW, D), np.float32)
    saug[fidx, cols % D] = 1.0

    woaug = np.zeros((XW, OUT), np.float32)
    woaug[np.arange(F) * AUGW + D] = wo[0]  # e1d cols -> fm1_sparse * wo0
    woaug[np.arange(F) * AUGW + D + 1] = -0.5 * wo[1]  # sq cols -> -0.5*t2*wo1
    woc2 = np.zeros((45, OUT), np.float32)
    woc2[0:32] = woaug[256:288]
    woc2[32:45] = dense_w.reshape(FN, 1) @ wo[0:1]

    blocks = dict(
        w1c0=w1aug[0:128], w1c1=w1aug[128:256], w1c2=w1c2,
        sc0=saug[0:128], sc1=saug[128:256], sc2=saug[256:288],
        w2c0=w2[0:128], w2c1=w2[128:256],
        woc0=woaug[0:128], woc1=woaug[128:256], woc2=woc2,
        wsq=np.tile(0.5 * wo[1:2], (D, 1)),
        wo2=wo[2:130],
    )
    wpack = np.zeros((128, WPACK_COLS), BF16NP)
    for name, (off, ncols, nrows) in WOFFS.items():
        arr = blocks[name]
        assert arr.shape == (nrows, ncols), (name, arr.shape, (nrows, ncols))
        wpack[0:nrows, off : off + ncols] = arr.astype(BF16NP)
    biasd = np.zeros((128, 4), np.float32)
    biasd[:, 0] = b1[0:128]
    biasd[:, 1] = b1[128:256]
    biasd[:, 2] = b2
    biasd[:, 3] = bo + dense_b[0] * wo[0]
    return dict(
        padbf=padbf,
        wpack=wpack,
        biasd=biasd,
        identd=np.eye(128, dtype=BF16NP),
    )


def make_core_inputs(shared, cf, nflat, rlo, nblk):
    r_rows = nblk * BLK_ROWS
    # idx16[:, f, b, :]: idx j (= local row c*128+p of block b) wrapped at
    # [j % 16, j // 16], replicated across the 8 gpsimd cores (16-row tiles).
    cb = cf[rlo : rlo + r_rows].reshape(nblk, BLK_ROWS, F)  # [b, j, f]
    w16 = cb.transpose(2, 0, 1).reshape(F, nblk, 128, 16).transpose(0, 1, 3, 2)
    idxk = np.tile(w16, (1, 1, 8, 1)).astype(np.int16)  # [F, nblk, 128, 128]
    idxk = np.ascontiguousarray(idxk.transpose(2, 0, 1, 3))  # [128, F, nblk, 128]
    nTk = np.ascontiguousarray(nflat[rlo : rlo + r_rows].T.astype(BF16NP))
    m = dict(shared)
    m["idx16"] = idxk
    m["idx0"] = np.ascontiguousarray(idxk[:, :, 0, :])
    m["nT"] = nTk
    return m


_NC_CACHE = {}
_RUN_KWARGS = {}


def kernel(**inputs):
    n_features = np.asarray(inputs["n_features"], np.float32)
    c_features = np.asarray(inputs["c_features"])

    shared = prep_weights(inputs)
    cf = c_features.reshape(ROWS, F).astype(np.int32)
    nflat = n_features.reshape(ROWS, FN)

    nblk = R // BLK_ROWS
    if nblk not in _NC_CACHE:
        _NC_CACHE[nblk] = build_program(nblk)
    nc = _NC_CACHE[nblk]

    in_maps = [
        make_core_inputs(shared, cf, nflat, k * R, nblk) for k in range(NCORES)
    ]
    kres = run_bass_kernel_spmd(nc, in_maps, list(range(NCORES)), **_RUN_KWARGS)
    kernel.last_results = kres
    res = kres.results

    out = np.empty((ROWS, OUT), np.float32)
    for k in range(NCORES):
        out[k * R : (k + 1) * R] = res[k]["outT"].T
    return out.reshape(B, G, OUT)


if __name__ == "__main__":
    print("building program...")
    nc = build_program(R // BLK_ROWS)
    print("ok")
